# revision 18
# baseline (speedup 1.0000x reference)
"""Causal attention with ALiBi for nn_CausalAttention (B=4, T=2048, C=1024,
16 heads) on 8 TRN2 NeuronCores.

Sharding: batch (4) x head-group (2 groups of 8 heads) -> 8 cores, with
heads interleaved even/odd across the two groups so that head-slot s holds
original heads (2s, 2s+1) on groups (0, 1). ALiBi slopes decay
geometrically with head index, so slot s only needs keys within a window
W_s = 16 * 2^(s+1) positions back (contributions beyond are < e^-16
relative); score tiles outside the window are skipped entirely.

Per core (one batch b, one head group g), phased for continuous PE
streaming:
  A: load x -> bf16 SBUF (once), weights -> bf16 SBUF.
  B: v projection -> vaug [j, slot, hd+ones]; q/k projections in [d, t]
     layout via PSUM->bf16 cast + SBUF-to-SBUF DMA into qT2/kT2 (rows
     64-71 hold augmented ALiBi rows: kaug one-hot per slot, qaug
     -slope*i which cancels per-row in softmax; K=72 contracts both).
  C: per slot, per key-block jb: QK matmuls over the windowed i-chunks
     (diagonal chunk narrowed to skip fully-masked columns), exp via ACT
     with bias +slope*j (exact fp32), causal mask by gpsimd affine_select
     on the [128,128] diagonal strip only (also kills Inf); after each
     4th jb, PV for the completed i-chunk with an appended ones column
     for the softmax denominator; normalize via vector.reciprocal +
     gpsimd partition_broadcast into oT.
  D: y = oT.T @ Wo_rows per t-block.
Host sums the two head-group partials per batch.
"""

import math

import numpy as np

import concourse.bass as bass
import concourse.mybir as mybir
import concourse.tile as tile
from concourse import bacc
from concourse.bass_utils import run_bass_kernel_spmd

B, T, C = 4, 2048, 1024
NH, HD = 16, 64
NHC = 8  # head-slots per core
NJB = T // 128  # 16 key blocks
NCH = T // 512  # 4 query chunks
P = 128

f32 = mybir.dt.float32
bf16 = mybir.dt.bfloat16

# per-slot attention window (keys further back contribute < e^-16 rel):
# slot s holds original heads (2s, 2s+1); binding slope = 2^-(s+1).
WIN = [32, 64, 128, 256, 512, 1024, 2048, 4096]

# last i-chunk covered by (slot, jb): include chunk c iff its first query
# can see block jb: 512c <= 128jb + 127 + W.
C1 = [
    [min(NCH - 1, (128 * jb + 127 + WIN[s]) // 512) for jb in range(NJB)]
    for s in range(NHC)
]
# first key block contributing to chunk c (same inequality, inverted)
JBMIN = [[0] * NCH for _ in range(NHC)]
for _s in range(NHC):
    for _c in range(NCH):
        _jm = 0
        while C1[_s][_jm] < _c:
            _jm += 1
        JBMIN[_s][_c] = _jm

LAST_RESULTS = None
_NC_CACHE = None


def get_slopes(n):
    def pow2(n):
        start = 2 ** (-(2 ** (-(math.log2(n) - 3))))
        return [start * start**i for i in range(n)]

    if math.log2(n).is_integer():
        return pow2(n)
    c = 2 ** math.floor(math.log2(n))
    return pow2(c) + get_slopes(2 * c)[0::2][: n - c]


def _select_diag(nc, ap):
    """causal mask on a [128, 128] diagonal strip: keep col - part >= 0."""
    nc.gpsimd.affine_select(
        ap,
        ap,
        pattern=[[1, 128]],
        compare_op=mybir.AluOpType.is_ge,
        fill=0.0,
        base=0,
        channel_multiplier=-1,
    )


def _emit_norm(nc, npool, oT, s, pot, c):
    """softmax denominator: row 64 of pot; normalize rows 0..63 into oT."""
    hp = (s % 2) * 64
    hm = s // 2
    potsb = npool.tile([65, 512], f32, tag="potsb")
    nc.vector.tensor_copy(potsb[:], pot[:])
    # spread the 512 rowsums across 128 partitions for a fast reciprocal
    rs128 = npool.tile([P, 4], f32, tag="rs")
    nc.sync.dma_start(rs128[:], potsb[64:65, :])
    nc.vector.reciprocal(rs128[:], rs128[:])
    srecip = npool.tile([1, 512], f32, tag="sr")
    nc.sync.dma_start(srecip[:], rs128[:])
    bcast = npool.tile([64, 512], f32, tag="bc")
    nc.gpsimd.partition_broadcast(bcast[:], srecip[:])
    nc.vector.tensor_tensor(
        oT[hp : hp + 64, hm, bass.ts(c, 512)],
        potsb[0:64, :],
        bcast[:],
        mybir.AluOpType.mult,
    )


def build_kernel():
    nc = bacc.Bacc("TRN2", target_bir_lowering=False, debug=False, num_devices=8)

    xT_d = nc.dram_tensor("xT", [C, T], f32, kind="ExternalInput").ap()
    wq_d = nc.dram_tensor("wq", [C, 512], f32, kind="ExternalInput").ap()
    wk_d = nc.dram_tensor("wk", [C, 512], f32, kind="ExternalInput").ap()
    wv_d = nc.dram_tensor("wv", [C, 512], f32, kind="ExternalInput").ap()
    wo_d = nc.dram_tensor("wo", [512, C], f32, kind="ExternalInput").ap()
    qaug_d = nc.dram_tensor("qaugb", [8, NHC, T], bf16, kind="ExternalInput").ap()
    kaug_d = nc.dram_tensor("kaugb", [8, NHC, T], bf16, kind="ExternalInput").ap()
    biasj_d = nc.dram_tensor("biasj", [P, NHC, NJB], f32, kind="ExternalInput").ap()
    y_d = nc.dram_tensor("y", [T, C], f32, kind="ExternalOutput").ap()

    xT_r = xT_d.rearrange("(cb p) t -> p cb t", p=P)  # [128, 8, 2048]
    wq_r = wq_d.rearrange("(cb p) m -> p cb m", p=P)  # [128, 8, 512]
    wk_r = wk_d.rearrange("(cb p) m -> p cb m", p=P)
    wv_r = wv_d.rearrange("(cb p) m -> p cb m", p=P)
    # [128, 4, 2, 512]: (pair m, 512-col half cc)
    wo_r = wo_d.rearrange("(mb p) (a n) -> p mb a n", p=P, a=2)
    y_r = y_d.rearrange("(tb p) c -> p tb c", p=P)  # [128, 16, 1024]

    with tile.TileContext(nc) as tc:
        with (
            tc.tile_pool(name="persist", bufs=1) as persist,
            tc.tile_pool(name="wpool", bufs=1) as wpool,
        ):
            vaug = persist.tile([P, NJB, NHC, 66], bf16)
            oT = persist.tile([P, 4, T], bf16)
            biasj = persist.tile([P, NHC, NJB], f32)
            wob = persist.tile([P, 4, 2, 512], bf16)
            wvb = wpool.tile([P, 8, 512], bf16, tag="wvb")
            wqb = wpool.tile([P, 8, 512], bf16, tag="wqb")
            wkb = wpool.tile([P, 8, 512], bf16, tag="wkb")

            nc.gpsimd.memset(vaug[:, :, :, 64:66], 1.0)
            nc.sync.dma_start(biasj[:], biasj_d[:])

            # ---- weight loads (scoped so the f32 staging frees early) ----
            with tc.tile_pool(name="st32", bufs=2) as st32:
                for wsrc, wdst in ((wq_r, wqb), (wk_r, wkb), (wv_r, wvb)):
                    for half in range(2):
                        w32 = st32.tile([P, 4, 512], f32, tag="w32")
                        nc.sync.dma_start(w32[:], wsrc[:, 4 * half : 4 * half + 4, :])
                        nc.vector.tensor_copy(
                            wdst[:, 4 * half : 4 * half + 4, :], w32[:]
                        )
                for half in range(2):
                    wo32 = st32.tile([P, 4, 512], f32, tag="w32")
                    nc.sync.dma_start(
                        wo32[:].rearrange("p (a b) n -> p a b n", a=2),
                        wo_r[:, 2 * half : 2 * half + 2, :, :],
                    )
                    nc.vector.tensor_copy(
                        wob[:, 2 * half : 2 * half + 2, :, :],
                        wo32[:].rearrange("p (a b) n -> p a b n", a=2),
                    )

            # ---- interleaved projections + attention ----
            # Projection work (v and q/k pairs) is emitted as fill items
            # inside the attention jb loop so the PE streams projection
            # chains while the ACT engine catches up on exponentials.
            with (
                tc.tile_pool(name="xstage", bufs=1) as xstage,
                tc.tile_pool(name="qkst", bufs=2) as qkst,
                tc.tile_pool(name="qkT", bufs=2) as qkT,
                tc.tile_pool(name="ptd_p", bufs=10) as ptd_p,
                tc.tile_pool(name="pt1_p", bufs=12) as pt1_p,
                tc.tile_pool(name="pt2_p", bufs=11) as pt2_p,
                tc.tile_pool(name="npool", bufs=3) as npool,
                tc.tile_pool(name="psP", bufs=2, space="PSUM") as psP,
                tc.tile_pool(name="psD", bufs=3, space="PSUM") as psD,
                tc.tile_pool(name="psA2", bufs=1, space="PSUM") as psA2,
                tc.tile_pool(name="psC", bufs=1, space="PSUM") as psC,
            ):
                def load_x_chunk(tck):
                    xtr = xstage.tile([P, 8, 512], bf16, tag="xtr", bufs=2)
                    for cb in range(8):
                        x32 = xstage.tile([P, 512], f32, tag="x32", bufs=6)
                        nc.sync.dma_start(x32[:], xT_r[:, cb, bass.ts(tck, 512)])
                        nc.vector.tensor_copy(xtr[:, cb, :], x32[:])
                    return xtr

                def v_chain(xtr, tck, tb):
                    psv = psP.tile([P, 512], f32, tag="pb")
                    for c in range(8):
                        nc.tensor.matmul(
                            psv[:],
                            xtr[:, c, bass.ts(tb, P)],
                            wvb[:, c, :],
                            start=(c == 0),
                            stop=(c == 7),
                        )
                    nc.vector.tensor_copy(
                        vaug[:, 4 * tck + tb, :, 0:64],
                        psv[:].rearrange("p (h d) -> p h d", h=NHC),
                    )

                def qk_chain(xtr, m, tck, qsg, ksg):
                    psq = psP.tile([P, 512], f32, tag="pb")
                    psk = psP.tile([P, 512], f32, tag="pb")
                    for c in range(8):
                        nc.tensor.matmul(
                            psq[:],
                            wqb[:, c, bass.ts(m, P)],
                            xtr[:, c, :],
                            start=(c == 0),
                            stop=(c == 7),
                        )
                        nc.tensor.matmul(
                            psk[:],
                            wkb[:, c, bass.ts(m, P)],
                            xtr[:, c, :],
                            start=(c == 0),
                            stop=(c == 7),
                        )
                    nc.vector.tensor_copy(qsg[:, bass.ts(tck, 512)], psq[:])
                    nc.vector.tensor_copy(ksg[:, bass.ts(tck, 512)], psk[:])

                pair_tiles = {}
                pair_state = {}

                def start_pair(m):
                    qTt = qkT.tile([72, 2, T], bf16, tag="qT")
                    kTt = qkT.tile([72, 2, T], bf16, tag="kT")
                    nc.sync.dma_start(qTt[64:72, :, :], qaug_d[:, 2 * m : 2 * m + 2, :])
                    nc.sync.dma_start(kTt[64:72, :, :], kaug_d[:, 2 * m : 2 * m + 2, :])
                    qsg = qkst.tile([P, T], bf16, tag="qsg")
                    ksg = qkst.tile([P, T], bf16, tag="ksg")
                    pair_tiles[m] = (qTt, kTt)
                    pair_state[m] = (qsg, ksg)

                def pair_item(m, tck):
                    if tck == 0:
                        start_pair(m)
                    qsg, ksg = pair_state[m]
                    xtr = load_x_chunk(tck)
                    qk_chain(xtr, m, tck, qsg, ksg)
                    if tck == 3:
                        qTt, kTt = pair_tiles[m]
                        nc.sync.dma_start(qTt[0:64, 0, :], qsg[0:64, :])
                        nc.sync.dma_start(qTt[0:64, 1, :], qsg[64:128, :])
                        nc.sync.dma_start(kTt[0:64, 0, :], ksg[0:64, :])
                        nc.sync.dma_start(kTt[0:64, 1, :], ksg[64:128, :])

                # pair 0 projected up front (slot 0 needs it immediately)
                for tck in range(NCH):
                    pair_item(0, tck)

                # fill items per slot: v projection streams through slot 0
                # (PV of chunk c only needs v blocks <= 4c+3); pair m's
                # projections stream through earlier slots.
                _vx = {}

                def v_item(tck, tb):
                    if tb == 0:
                        _vx[tck] = load_x_chunk(tck)
                    v_chain(_vx[tck], tck, tb)

                fills = {s: [] for s in range(NHC)}
                fills[0] = [
                    (tb_ + 4 * tck_, lambda tck=tck_, tb=tb_: v_item(tck, tb))
                    for tck_ in range(NCH)
                    for tb_ in range(4)
                ]
                fills[1] = [
                    (4 * tck_, lambda tck=tck_: pair_item(1, tck))
                    for tck_ in range(NCH)
                ]
                for pm in (2, 3):
                    for tck_ in range(NCH):
                        sl = 2 * (pm - 1) + tck_ // 2
                        fills[sl].append(
                            (8 * (tck_ % 2), lambda tck=tck_, m=pm: pair_item(m, tck))
                        )

                # (s, jb, c) -> (tile, mid_idx or None, width, pot col offset)
                pt_reg = {}
                pend_pv = []  # chunks whose PV is deferred one jb step
                pend_norm = []  # pots whose normalization is deferred one more

                def emit_pv(s, c):
                    jmin = JBMIN[s][c]
                    njb = 4 * c + 4 - jmin
                    pot = psC.tile([65, 512], f32, tag="pot")
                    for idx, jbp in enumerate(range(jmin, 4 * c + 4)):
                        tl, mi, w, off = pt_reg.pop((s, jbp, c))
                        mov = tl[:, 0:w] if mi is None else tl[:, mi, 0:w]
                        nc.tensor.matmul(
                            pot[:, off : off + w],
                            vaug[:, jbp, s, 0:65],
                            mov,
                            start=(idx == 0),
                            stop=(idx == njb - 1),
                        )
                    pend_norm.append((s, pot, c))

                for s in range(NHC):
                    qTt, kTt = pair_tiles[s // 2]
                    sp = s % 2
                    slot_fills = list(fills[s])
                    for jb in range(NJB):
                        c0 = jb // 4
                        r = jb % 4
                        c1 = C1[s][jb]
                        if r > 0:
                            # diagonal chunk, narrowed: cols 128r..512 of c0
                            w = 512 - 128 * r
                            sd = psD.tile([P, 512], f32, tag="sd")
                            nc.tensor.matmul(
                                sd[:, 0:w],
                                kTt[:, sp, bass.ts(jb, P)],
                                qTt[:, sp, 512 * c0 + 128 * r : 512 * (c0 + 1)],
                                start=True,
                                stop=True,
                            )
                            td = ptd_p.tile([P, 512], bf16, tag="ptd")
                            nc.scalar.activation(
                                td[:, 0:w],
                                sd[:, 0:w],
                                mybir.ActivationFunctionType.Exp,
                                bias=biasj[:, s, jb : jb + 1],
                                scale=1.0,
                            )
                            _select_diag(nc, td[:, 0:128])
                            pt_reg[(s, jb, c0)] = (td, None, w, 128 * r)
                            fulls = list(range(c0 + 1, c1 + 1))
                        else:
                            fulls = list(range(c0, c1 + 1))

                        g = 0
                        while g < len(fulls):
                            ng = min(2, len(fulls) - g)
                            if ng == 2:
                                s2 = psA2.tile([P, 2, 512], f32, tag="sa")
                                t2 = pt2_p.tile([P, 2, 512], bf16, tag="pt2")
                                for i in range(2):
                                    nc.tensor.matmul(
                                        s2[:, i, :],
                                        kTt[:, sp, bass.ts(jb, P)],
                                        qTt[:, sp, bass.ts(fulls[g + i], 512)],
                                        start=True,
                                        stop=True,
                                    )
                                nc.scalar.activation(
                                    t2[:],
                                    s2[:],
                                    mybir.ActivationFunctionType.Exp,
                                    bias=biasj[:, s, jb : jb + 1],
                                    scale=1.0,
                                )
                                for i in range(2):
                                    pt_reg[(s, jb, fulls[g + i])] = (t2, i, 512, 0)
                            else:
                                s1 = psD.tile([P, 512], f32, tag="sd")
                                t1 = pt1_p.tile([P, 512], bf16, tag="pt1")
                                nc.tensor.matmul(
                                    s1[:],
                                    kTt[:, sp, bass.ts(jb, P)],
                                    qTt[:, sp, bass.ts(fulls[g], 512)],
                                    start=True,
                                    stop=True,
                                )
                                nc.scalar.activation(
                                    t1[:],
                                    s1[:],
                                    mybir.ActivationFunctionType.Exp,
                                    bias=biasj[:, s, jb : jb + 1],
                                    scale=1.0,
                                )
                                pt_reg[(s, jb, fulls[g])] = (t1, None, 512, 0)
                            g += ng

                        if r == 0:
                            tl, mi, _, _ = pt_reg[(s, jb, c0)]
                            sel_ap = tl[:, 0:128] if mi is None else tl[:, mi, 0:128]
                            _select_diag(nc, sel_ap)

                        # projection fill items scheduled for this step: PE
                        # work that hides the exp/select latency of this jb
                        # before the deferred PV below needs it.
                        while slot_fills and slot_fills[0][0] <= jb:
                            slot_fills.pop(0)[1]()

                        # flush work deferred from the previous jb step
                        if pend_norm:
                            ns_, pot_, nc_ = pend_norm.pop(0)
                            _emit_norm(nc, npool, oT, ns_, pot_, nc_)
                        if pend_pv:
                            emit_pv(*pend_pv.pop(0))
                        if (jb + 1) % 4 == 0:
                            pend_pv.append((s, jb // 4))

                    while slot_fills:
                        slot_fills.pop(0)[1]()

                # drain the tail
                while pend_pv or pend_norm:
                    if pend_norm:
                        ns_, pot_, nc_ = pend_norm.pop(0)
                        _emit_norm(nc, npool, oT, ns_, pot_, nc_)
                    if pend_pv:
                        emit_pv(*pend_pv.pop(0))

            # ---- phase D: output projection ----
            with (
                tc.tile_pool(name="ypool", bufs=4) as ypool,
                tc.tile_pool(name="psY", bufs=4, space="PSUM") as psY,
            ):
                for tb in range(NJB):
                    for cc in range(2):
                        psy = psY.tile([P, 512], f32, tag="py")
                        for m in range(4):
                            nc.tensor.matmul(
                                psy[:],
                                oT[:, m, bass.ts(tb, P)],
                                wob[:, m, cc, :],
                                start=(m == 0),
                                stop=(m == 3),
                            )
                        ysb = ypool.tile([P, 512], f32, tag="ysb")
                        nc.vector.tensor_copy(ysb[:], psy[:])
                        nc.sync.dma_start(y_r[:, tb, bass.ts(cc, 512)], ysb[:])

    nc.compile()
    return nc


def kernel(x, Wq, Wk, Wv, Wo):
    global LAST_RESULTS, _NC_CACHE
    import ml_dtypes

    x = np.asarray(x, dtype=np.float32)
    Wq = np.asarray(Wq, dtype=np.float32)
    Wk = np.asarray(Wk, dtype=np.float32)
    Wv = np.asarray(Wv, dtype=np.float32)
    Wo = np.asarray(Wo, dtype=np.float32)

    slopes = np.asarray(get_slopes(NH), dtype=np.float64)
    ii = np.arange(T, dtype=np.float64)
    pp = np.arange(P, dtype=np.float64)

    if _NC_CACHE is None:
        _NC_CACHE = build_kernel()
    nc = _NC_CACHE

    in_maps = []
    for core in range(8):
        b, g = core // 2, core % 2
        perm = list(range(g, NH, 2))  # slot s -> original head 2s+g
        core_slopes = slopes[perm]

        qaug1 = (-core_slopes[:, None] * ii[None, :]).astype(ml_dtypes.bfloat16)
        qaugb = np.ascontiguousarray(np.broadcast_to(qaug1[:, None, :], (8, NHC, T)))
        kaugb = np.zeros((8, NHC, T), ml_dtypes.bfloat16)
        for h in range(NHC):
            kaugb[h, h, :] = ml_dtypes.bfloat16(1.0)
        biasj = np.zeros((P, NHC, NJB), np.float32)
        for h in range(NHC):
            for jb in range(NJB):
                biasj[:, h, jb] = (core_slopes[h] * (128 * jb + pp)).astype(np.float32)

        wq_g = np.concatenate([Wq[:, 64 * h : 64 * h + 64] for h in perm], axis=1)
        wk_g = np.concatenate([Wk[:, 64 * h : 64 * h + 64] for h in perm], axis=1)
        wv_g = np.concatenate([Wv[:, 64 * h : 64 * h + 64] for h in perm], axis=1)
        wo_g = np.concatenate([Wo[64 * h : 64 * h + 64, :] for h in perm], axis=0)

        in_maps.append(
            {
                "xT": np.ascontiguousarray(x[b].T),
                "wq": np.ascontiguousarray(wq_g) * np.float32(0.125),
                "wk": np.ascontiguousarray(wk_g),
                "wv": np.ascontiguousarray(wv_g),
                "wo": np.ascontiguousarray(wo_g),
                "qaugb": qaugb,
                "kaugb": kaugb,
                "biasj": biasj,
            }
        )

    res = run_bass_kernel_spmd(nc, in_maps, list(range(8)))
    LAST_RESULTS = res
    out = np.empty((B, T, C), dtype=np.float32)
    for b in range(B):
        out[b] = res.results[2 * b]["y"] + res.results[2 * b + 1]["y"]
    return out


# revision 19
# speedup vs baseline: 1.1184x; 1.1184x over previous
"""Causal attention with ALiBi for nn_CausalAttention (B=4, T=2048, C=1024,
16 heads) on 8 TRN2 NeuronCores.

Sharding: batch (4) x head-group (2 groups of 8 heads) -> 8 cores, with
heads interleaved even/odd across the two groups so that head-slot s holds
original heads (2s, 2s+1) on groups (0, 1). ALiBi slopes decay
geometrically with head index, so slot s only needs keys within a window
W_s = 16 * 2^(s+1) positions back (contributions beyond are < e^-16
relative); score tiles outside the window are skipped entirely.

Per core (one batch b, one head group g), phased for continuous PE
streaming:
  A: load x -> bf16 SBUF (once), weights -> bf16 SBUF.
  B: v projection -> vaug [j, slot, hd+ones]; q/k projections in [d, t]
     layout via PSUM->bf16 cast + SBUF-to-SBUF DMA into qT2/kT2 (rows
     64-71 hold augmented ALiBi rows: kaug one-hot per slot, qaug
     -slope*i which cancels per-row in softmax; K=72 contracts both).
  C: per slot, per key-block jb: QK matmuls over the windowed i-chunks
     (diagonal chunk narrowed to skip fully-masked columns), exp via ACT
     with bias +slope*j (exact fp32), causal mask by gpsimd affine_select
     on the [128,128] diagonal strip only (also kills Inf); after each
     4th jb, PV for the completed i-chunk with an appended ones column
     for the softmax denominator; normalize via vector.reciprocal +
     gpsimd partition_broadcast into oT.
  D: y = oT.T @ Wo_rows per t-block.
Host sums the two head-group partials per batch.
"""

import math

import numpy as np

import concourse.bass as bass
import concourse.mybir as mybir
import concourse.tile as tile
from concourse import bacc
from concourse.bass_utils import run_bass_kernel_spmd

B, T, C = 4, 2048, 1024
NH, HD = 16, 64
NHC = 8  # head-slots per core
NJB = T // 128  # 16 key blocks
NCH = T // 512  # 4 query chunks
P = 128

f32 = mybir.dt.float32
bf16 = mybir.dt.bfloat16

# per-slot attention window (keys further back contribute < e^-16 rel):
# slot s holds original heads (2s, 2s+1); binding slope = 2^-(s+1).
WIN = [32, 64, 128, 256, 512, 1024, 2048, 4096]

# last i-chunk covered by (slot, jb): include chunk c iff its first query
# can see block jb: 512c <= 128jb + 127 + W.
C1 = [
    [min(NCH - 1, (128 * jb + 127 + WIN[s]) // 512) for jb in range(NJB)]
    for s in range(NHC)
]
# first key block contributing to chunk c (same inequality, inverted)
JBMIN = [[0] * NCH for _ in range(NHC)]
for _s in range(NHC):
    for _c in range(NCH):
        _jm = 0
        while C1[_s][_jm] < _c:
            _jm += 1
        JBMIN[_s][_c] = _jm

LAST_RESULTS = None
_NC_CACHE = None


def get_slopes(n):
    def pow2(n):
        start = 2 ** (-(2 ** (-(math.log2(n) - 3))))
        return [start * start**i for i in range(n)]

    if math.log2(n).is_integer():
        return pow2(n)
    c = 2 ** math.floor(math.log2(n))
    return pow2(c) + get_slopes(2 * c)[0::2][: n - c]


def _select_diag(nc, ap):
    """causal mask on a [128, 128] diagonal strip: keep col - part >= 0."""
    nc.gpsimd.affine_select(
        ap,
        ap,
        pattern=[[1, 128]],
        compare_op=mybir.AluOpType.is_ge,
        fill=0.0,
        base=0,
        channel_multiplier=-1,
    )


def _emit_norm(nc, npool, oT, s, pot, c):
    """softmax denominator: row 64 of pot; normalize rows 0..63 into oT."""
    hp = (s % 2) * 64
    hm = s // 2
    potsb = npool.tile([65, 512], f32, tag="potsb")
    nc.vector.tensor_copy(potsb[:], pot[:])
    # spread the 512 rowsums across 128 partitions for a fast reciprocal
    rs128 = npool.tile([P, 4], f32, tag="rs")
    nc.sync.dma_start(rs128[:], potsb[64:65, :])
    nc.vector.reciprocal(rs128[:], rs128[:])
    srecip = npool.tile([1, 512], f32, tag="sr")
    nc.sync.dma_start(srecip[:], rs128[:])
    bcast = npool.tile([64, 512], f32, tag="bc")
    nc.gpsimd.partition_broadcast(bcast[:], srecip[:])
    nc.vector.tensor_tensor(
        oT[hp : hp + 64, hm, bass.ts(c, 512)],
        potsb[0:64, :],
        bcast[:],
        mybir.AluOpType.mult,
    )


def build_kernel():
    nc = bacc.Bacc("TRN2", target_bir_lowering=False, debug=False, num_devices=8)

    xT_d = nc.dram_tensor("xT", [C, T], f32, kind="ExternalInput").ap()
    wq_d = nc.dram_tensor("wq", [C, 512], f32, kind="ExternalInput").ap()
    wk_d = nc.dram_tensor("wk", [C, 512], f32, kind="ExternalInput").ap()
    wv_d = nc.dram_tensor("wv", [C, 512], f32, kind="ExternalInput").ap()
    wo_d = nc.dram_tensor("wo", [512, C], f32, kind="ExternalInput").ap()
    qaug_d = nc.dram_tensor("qaugb", [8, NHC, T], bf16, kind="ExternalInput").ap()
    kaug_d = nc.dram_tensor("kaugb", [8, NHC, T], bf16, kind="ExternalInput").ap()
    biasj_d = nc.dram_tensor("biasj", [P, NHC, NJB], f32, kind="ExternalInput").ap()
    y_d = nc.dram_tensor("y", [T, C], f32, kind="ExternalOutput").ap()

    xT_r = xT_d.rearrange("(cb p) t -> p cb t", p=P)  # [128, 8, 2048]
    wq_r = wq_d.rearrange("(cb p) m -> p cb m", p=P)  # [128, 8, 512]
    wk_r = wk_d.rearrange("(cb p) m -> p cb m", p=P)
    wv_r = wv_d.rearrange("(cb p) m -> p cb m", p=P)
    # [128, 4, 2, 512]: (pair m, 512-col half cc)
    wo_r = wo_d.rearrange("(mb p) (a n) -> p mb a n", p=P, a=2)
    y_r = y_d.rearrange("(tb p) c -> p tb c", p=P)  # [128, 16, 1024]

    with tile.TileContext(nc) as tc:
        with (
            tc.tile_pool(name="persist", bufs=1) as persist,
            tc.tile_pool(name="wpool", bufs=1) as wpool,
        ):
            vaug = persist.tile([P, NJB, NHC, 66], bf16)
            oT = persist.tile([P, 4, T], bf16)
            biasj = persist.tile([P, NHC, NJB], f32)
            wob = persist.tile([P, 4, 2, 512], bf16)
            xb = wpool.tile([P, 8, T], bf16, tag="xb")
            wvb = wpool.tile([P, 8, 512], bf16, tag="wvb")
            wqb = wpool.tile([P, 8, 512], bf16, tag="wqb")
            wkb = wpool.tile([P, 8, 512], bf16, tag="wkb")

            nc.gpsimd.memset(vaug[:, :, :, 64:66], 1.0)
            nc.sync.dma_start(biasj[:], biasj_d[:])

            # ---- x + weight loads (scoped so the f32 staging frees early) ----
            with tc.tile_pool(name="st32", bufs=2) as st32:
                for cb in range(8):
                    x32 = st32.tile([P, T], f32, tag="x32", bufs=3)
                    nc.sync.dma_start(x32[:], xT_r[:, cb, :])
                    nc.vector.tensor_copy(xb[:, cb, :], x32[:])
                for wsrc, wdst in ((wq_r, wqb), (wk_r, wkb), (wv_r, wvb)):
                    for half in range(2):
                        w32 = st32.tile([P, 4, 512], f32, tag="w32")
                        nc.sync.dma_start(w32[:], wsrc[:, 4 * half : 4 * half + 4, :])
                        nc.vector.tensor_copy(
                            wdst[:, 4 * half : 4 * half + 4, :], w32[:]
                        )
                for half in range(2):
                    wo32 = st32.tile([P, 4, 512], f32, tag="w32")
                    nc.sync.dma_start(
                        wo32[:].rearrange("p (a b) n -> p a b n", a=2),
                        wo_r[:, 2 * half : 2 * half + 2, :, :],
                    )
                    nc.vector.tensor_copy(
                        wob[:, 2 * half : 2 * half + 2, :, :],
                        wo32[:].rearrange("p (a b) n -> p a b n", a=2),
                    )

            # ---- interleaved projections + attention ----
            # Projection work (v and q/k pairs) is emitted as fill items
            # inside the attention jb loop so the PE streams projection
            # chains while the ACT engine catches up on exponentials.
            with (
                tc.tile_pool(name="qkst", bufs=2) as qkst,
                tc.tile_pool(name="qkT", bufs=2) as qkT,
                tc.tile_pool(name="ptd_p", bufs=8) as ptd_p,
                tc.tile_pool(name="pt1_p", bufs=10) as pt1_p,
                tc.tile_pool(name="pt2_p", bufs=10) as pt2_p,
                tc.tile_pool(name="npool", bufs=2) as npool,
                tc.tile_pool(name="psP", bufs=2, space="PSUM") as psP,
                tc.tile_pool(name="psD", bufs=3, space="PSUM") as psD,
                tc.tile_pool(name="psA2", bufs=1, space="PSUM") as psA2,
                tc.tile_pool(name="psC", bufs=1, space="PSUM") as psC,
            ):
                def v_chain(tck, tb):
                    psv = psP.tile([P, 512], f32, tag="pb")
                    for c in range(8):
                        nc.tensor.matmul(
                            psv[:],
                            xb[:, c, 512 * tck + 128 * tb : 512 * tck + 128 * (tb + 1)],
                            wvb[:, c, :],
                            start=(c == 0),
                            stop=(c == 7),
                        )
                    nc.vector.tensor_copy(
                        vaug[:, 4 * tck + tb, :, 0:64],
                        psv[:].rearrange("p (h d) -> p h d", h=NHC),
                    )

                def qk_chain(m, tck, qsg, ksg):
                    psq = psP.tile([P, 512], f32, tag="pb")
                    psk = psP.tile([P, 512], f32, tag="pb")
                    for c in range(8):
                        nc.tensor.matmul(
                            psq[:],
                            wqb[:, c, bass.ts(m, P)],
                            xb[:, c, bass.ts(tck, 512)],
                            start=(c == 0),
                            stop=(c == 7),
                        )
                        nc.tensor.matmul(
                            psk[:],
                            wkb[:, c, bass.ts(m, P)],
                            xb[:, c, bass.ts(tck, 512)],
                            start=(c == 0),
                            stop=(c == 7),
                        )
                    nc.vector.tensor_copy(qsg[:, bass.ts(tck, 512)], psq[:])
                    nc.vector.tensor_copy(ksg[:, bass.ts(tck, 512)], psk[:])

                pair_tiles = {}
                pair_state = {}

                def start_pair(m):
                    qTt = qkT.tile([72, 2, T], bf16, tag="qT")
                    kTt = qkT.tile([72, 2, T], bf16, tag="kT")
                    nc.sync.dma_start(qTt[64:72, :, :], qaug_d[:, 2 * m : 2 * m + 2, :])
                    nc.sync.dma_start(kTt[64:72, :, :], kaug_d[:, 2 * m : 2 * m + 2, :])
                    qsg = qkst.tile([P, T], bf16, tag="qsg")
                    ksg = qkst.tile([P, T], bf16, tag="ksg")
                    pair_tiles[m] = (qTt, kTt)
                    pair_state[m] = (qsg, ksg)

                def pair_item(m, tck):
                    if tck == 0:
                        start_pair(m)
                    qsg, ksg = pair_state[m]
                    qk_chain(m, tck, qsg, ksg)
                    if tck == 3:
                        qTt, kTt = pair_tiles[m]
                        nc.sync.dma_start(qTt[0:64, 0, :], qsg[0:64, :])
                        nc.sync.dma_start(qTt[0:64, 1, :], qsg[64:128, :])
                        nc.sync.dma_start(kTt[0:64, 0, :], ksg[0:64, :])
                        nc.sync.dma_start(kTt[0:64, 1, :], ksg[64:128, :])

                # pair 0 projected up front (slot 0 needs it immediately)
                for tck in range(NCH):
                    pair_item(0, tck)

                # fill items per slot: v projection streams through slot 0
                # (PV of chunk c only needs v blocks <= 4c+3); pair m's
                # projections stream through earlier slots.
                def v_item(tck, tb):
                    v_chain(tck, tb)

                fills = {s: [] for s in range(NHC)}
                fills[0] = [
                    (tb_ + 4 * tck_, lambda tck=tck_, tb=tb_: v_item(tck, tb))
                    for tck_ in range(NCH)
                    for tb_ in range(4)
                ]
                fills[1] = [
                    (4 * tck_, lambda tck=tck_: pair_item(1, tck))
                    for tck_ in range(NCH)
                ]
                for pm in (2, 3):
                    for tck_ in range(NCH):
                        sl = 2 * (pm - 1) + tck_ // 2
                        fills[sl].append(
                            (8 * (tck_ % 2), lambda tck=tck_, m=pm: pair_item(m, tck))
                        )

                # (s, jb, c) -> (tile, mid_idx or None, width, pot col offset)
                pt_reg = {}
                pend_pv = []  # chunks whose PV is deferred one jb step
                pend_norm = []  # pots whose normalization is deferred one more

                def emit_pv(s, c):
                    jmin = JBMIN[s][c]
                    njb = 4 * c + 4 - jmin
                    pot = psC.tile([65, 512], f32, tag="pot")
                    for idx, jbp in enumerate(range(jmin, 4 * c + 4)):
                        tl, mi, w, off = pt_reg.pop((s, jbp, c))
                        mov = tl[:, 0:w] if mi is None else tl[:, mi, 0:w]
                        nc.tensor.matmul(
                            pot[:, off : off + w],
                            vaug[:, jbp, s, 0:65],
                            mov,
                            start=(idx == 0),
                            stop=(idx == njb - 1),
                        )
                    pend_norm.append((s, pot, c))

                for s in range(NHC):
                    qTt, kTt = pair_tiles[s // 2]
                    sp = s % 2
                    slot_fills = list(fills[s])
                    for jb in range(NJB):
                        # projection fill items first: PE work that gives
                        # the ACT engine headroom before this jb's QK needs
                        # score-PSUM buffers back.
                        while slot_fills and slot_fills[0][0] <= jb:
                            slot_fills.pop(0)[1]()

                        c0 = jb // 4
                        r = jb % 4
                        c1 = C1[s][jb]
                        if r > 0:
                            # diagonal chunk, narrowed: cols 128r..512 of c0
                            w = 512 - 128 * r
                            sd = psD.tile([P, 512], f32, tag="sd")
                            nc.tensor.matmul(
                                sd[:, 0:w],
                                kTt[:, sp, bass.ts(jb, P)],
                                qTt[:, sp, 512 * c0 + 128 * r : 512 * (c0 + 1)],
                                start=True,
                                stop=True,
                            )
                            td = ptd_p.tile([P, 512], bf16, tag="ptd")
                            nc.scalar.activation(
                                td[:, 0:w],
                                sd[:, 0:w],
                                mybir.ActivationFunctionType.Exp,
                                bias=biasj[:, s, jb : jb + 1],
                                scale=1.0,
                            )
                            _select_diag(nc, td[:, 0:128])
                            pt_reg[(s, jb, c0)] = (td, None, w, 128 * r)
                            fulls = list(range(c0 + 1, c1 + 1))
                        else:
                            fulls = list(range(c0, c1 + 1))

                        g = 0
                        while g < len(fulls):
                            ng = min(2, len(fulls) - g)
                            if ng == 2:
                                s2 = psA2.tile([P, 2, 512], f32, tag="sa")
                                t2 = pt2_p.tile([P, 2, 512], bf16, tag="pt2")
                                for i in range(2):
                                    nc.tensor.matmul(
                                        s2[:, i, :],
                                        kTt[:, sp, bass.ts(jb, P)],
                                        qTt[:, sp, bass.ts(fulls[g + i], 512)],
                                        start=True,
                                        stop=True,
                                    )
                                nc.scalar.activation(
                                    t2[:],
                                    s2[:],
                                    mybir.ActivationFunctionType.Exp,
                                    bias=biasj[:, s, jb : jb + 1],
                                    scale=1.0,
                                )
                                for i in range(2):
                                    pt_reg[(s, jb, fulls[g + i])] = (t2, i, 512, 0)
                            else:
                                s1 = psD.tile([P, 512], f32, tag="sd")
                                t1 = pt1_p.tile([P, 512], bf16, tag="pt1")
                                nc.tensor.matmul(
                                    s1[:],
                                    kTt[:, sp, bass.ts(jb, P)],
                                    qTt[:, sp, bass.ts(fulls[g], 512)],
                                    start=True,
                                    stop=True,
                                )
                                nc.scalar.activation(
                                    t1[:],
                                    s1[:],
                                    mybir.ActivationFunctionType.Exp,
                                    bias=biasj[:, s, jb : jb + 1],
                                    scale=1.0,
                                )
                                pt_reg[(s, jb, fulls[g])] = (t1, None, 512, 0)
                            g += ng

                        if r == 0:
                            tl, mi, _, _ = pt_reg[(s, jb, c0)]
                            sel_ap = tl[:, 0:128] if mi is None else tl[:, mi, 0:128]
                            _select_diag(nc, sel_ap)

                        # flush work deferred from the previous jb step
                        if pend_norm:
                            ns_, pot_, nc_ = pend_norm.pop(0)
                            _emit_norm(nc, npool, oT, ns_, pot_, nc_)
                        if pend_pv:
                            emit_pv(*pend_pv.pop(0))
                        if (jb + 1) % 4 == 0:
                            pend_pv.append((s, jb // 4))

                    while slot_fills:
                        slot_fills.pop(0)[1]()

                # drain the tail
                while pend_pv or pend_norm:
                    if pend_norm:
                        ns_, pot_, nc_ = pend_norm.pop(0)
                        _emit_norm(nc, npool, oT, ns_, pot_, nc_)
                    if pend_pv:
                        emit_pv(*pend_pv.pop(0))

            # ---- phase D: output projection ----
            with (
                tc.tile_pool(name="ypool", bufs=4) as ypool,
                tc.tile_pool(name="psY", bufs=4, space="PSUM") as psY,
            ):
                for tb in range(NJB):
                    for cc in range(2):
                        psy = psY.tile([P, 512], f32, tag="py")
                        for m in range(4):
                            nc.tensor.matmul(
                                psy[:],
                                oT[:, m, bass.ts(tb, P)],
                                wob[:, m, cc, :],
                                start=(m == 0),
                                stop=(m == 3),
                            )
                        ysb = ypool.tile([P, 512], f32, tag="ysb")
                        nc.vector.tensor_copy(ysb[:], psy[:])
                        nc.sync.dma_start(y_r[:, tb, bass.ts(cc, 512)], ysb[:])

    nc.compile()
    return nc


def kernel(x, Wq, Wk, Wv, Wo):
    global LAST_RESULTS, _NC_CACHE
    import ml_dtypes

    x = np.asarray(x, dtype=np.float32)
    Wq = np.asarray(Wq, dtype=np.float32)
    Wk = np.asarray(Wk, dtype=np.float32)
    Wv = np.asarray(Wv, dtype=np.float32)
    Wo = np.asarray(Wo, dtype=np.float32)

    slopes = np.asarray(get_slopes(NH), dtype=np.float64)
    ii = np.arange(T, dtype=np.float64)
    pp = np.arange(P, dtype=np.float64)

    if _NC_CACHE is None:
        _NC_CACHE = build_kernel()
    nc = _NC_CACHE

    in_maps = []
    for core in range(8):
        b, g = core // 2, core % 2
        perm = list(range(g, NH, 2))  # slot s -> original head 2s+g
        core_slopes = slopes[perm]

        qaug1 = (-core_slopes[:, None] * ii[None, :]).astype(ml_dtypes.bfloat16)
        qaugb = np.ascontiguousarray(np.broadcast_to(qaug1[:, None, :], (8, NHC, T)))
        kaugb = np.zeros((8, NHC, T), ml_dtypes.bfloat16)
        for h in range(NHC):
            kaugb[h, h, :] = ml_dtypes.bfloat16(1.0)
        biasj = np.zeros((P, NHC, NJB), np.float32)
        for h in range(NHC):
            for jb in range(NJB):
                biasj[:, h, jb] = (core_slopes[h] * (128 * jb + pp)).astype(np.float32)

        wq_g = np.concatenate([Wq[:, 64 * h : 64 * h + 64] for h in perm], axis=1)
        wk_g = np.concatenate([Wk[:, 64 * h : 64 * h + 64] for h in perm], axis=1)
        wv_g = np.concatenate([Wv[:, 64 * h : 64 * h + 64] for h in perm], axis=1)
        wo_g = np.concatenate([Wo[64 * h : 64 * h + 64, :] for h in perm], axis=0)

        in_maps.append(
            {
                "xT": np.ascontiguousarray(x[b].T),
                "wq": np.ascontiguousarray(wq_g) * np.float32(0.125),
                "wk": np.ascontiguousarray(wk_g),
                "wv": np.ascontiguousarray(wv_g),
                "wo": np.ascontiguousarray(wo_g),
                "qaugb": qaugb,
                "kaugb": kaugb,
                "biasj": biasj,
            }
        )

    res = run_bass_kernel_spmd(nc, in_maps, list(range(8)))
    LAST_RESULTS = res
    out = np.empty((B, T, C), dtype=np.float32)
    for b in range(B):
        out[b] = res.results[2 * b]["y"] + res.results[2 * b + 1]["y"]
    return out


# revision 20
# speedup vs baseline: 1.2206x; 1.0914x over previous
"""Causal attention with ALiBi for nn_CausalAttention (B=4, T=2048, C=1024,
16 heads) on 8 TRN2 NeuronCores.

Sharding: batch (4) x head-group (2 groups of 8 heads) -> 8 cores, with
heads interleaved even/odd across the two groups so that head-slot s holds
original heads (2s, 2s+1) on groups (0, 1). ALiBi slopes decay
geometrically with head index, so slot s only needs keys within a window
W_s = 16 * 2^(s+1) positions back (contributions beyond are < e^-16
relative); score tiles outside the window are skipped entirely.

Per core (one batch b, one head group g), phased for continuous PE
streaming:
  A: load x -> bf16 SBUF (once), weights -> bf16 SBUF.
  B: v projection -> vaug [j, slot, hd+ones]; q/k projections in [d, t]
     layout via PSUM->bf16 cast + SBUF-to-SBUF DMA into qT2/kT2 (rows
     64-71 hold augmented ALiBi rows: kaug one-hot per slot, qaug
     -slope*i which cancels per-row in softmax; K=72 contracts both).
  C: per slot, per key-block jb: QK matmuls over the windowed i-chunks
     (diagonal chunk narrowed to skip fully-masked columns), exp via ACT
     with bias +slope*j (exact fp32), causal mask by gpsimd affine_select
     on the [128,128] diagonal strip only (also kills Inf); after each
     4th jb, PV for the completed i-chunk with an appended ones column
     for the softmax denominator; normalize via vector.reciprocal +
     gpsimd partition_broadcast into oT.
  D: y = oT.T @ Wo_rows per t-block.
Host sums the two head-group partials per batch.
"""

import math

import numpy as np

import concourse.bass as bass
import concourse.mybir as mybir
import concourse.tile as tile
from concourse import bacc
from concourse.bass_utils import run_bass_kernel_spmd

B, T, C = 4, 2048, 1024
NH, HD = 16, 64
NHC = 8  # head-slots per core
NJB = T // 128  # 16 key blocks
NCH = T // 512  # 4 query chunks
P = 128

f32 = mybir.dt.float32
bf16 = mybir.dt.bfloat16

# per-slot attention window (keys further back contribute < e^-16 rel):
# slot s holds original heads (2s, 2s+1); binding slope = 2^-(s+1).
WIN = [32, 64, 128, 256, 512, 1024, 2048, 4096]

# last i-chunk covered by (slot, jb): include chunk c iff its first query
# can see block jb: 512c <= 128jb + 127 + W.
C1 = [
    [min(NCH - 1, (128 * jb + 127 + WIN[s]) // 512) for jb in range(NJB)]
    for s in range(NHC)
]
# first key block contributing to chunk c (same inequality, inverted)
JBMIN = [[0] * NCH for _ in range(NHC)]
for _s in range(NHC):
    for _c in range(NCH):
        _jm = 0
        while C1[_s][_jm] < _c:
            _jm += 1
        JBMIN[_s][_c] = _jm

LAST_RESULTS = None
_NC_CACHE = None


def get_slopes(n):
    def pow2(n):
        start = 2 ** (-(2 ** (-(math.log2(n) - 3))))
        return [start * start**i for i in range(n)]

    if math.log2(n).is_integer():
        return pow2(n)
    c = 2 ** math.floor(math.log2(n))
    return pow2(c) + get_slopes(2 * c)[0::2][: n - c]


def _select_diag(nc, ap):
    """causal mask on a [128, 128] diagonal strip: keep col - part >= 0."""
    nc.gpsimd.affine_select(
        ap,
        ap,
        pattern=[[1, 128]],
        compare_op=mybir.AluOpType.is_ge,
        fill=0.0,
        base=0,
        channel_multiplier=-1,
    )


def _emit_norm(nc, npool, oT, s, pot, c):
    """softmax denominator: row 64 of pot; normalize rows 0..63 into oT."""
    hp = (s % 2) * 64
    hm = s // 2
    potsb = npool.tile([65, 512], f32, tag="potsb")
    nc.vector.tensor_copy(potsb[:], pot[:])
    # spread the 512 rowsums across 128 partitions for a fast reciprocal
    rs128 = npool.tile([P, 4], f32, tag="rs")
    nc.sync.dma_start(rs128[:], potsb[64:65, :])
    nc.vector.reciprocal(rs128[:], rs128[:])
    srecip = npool.tile([1, 512], f32, tag="sr")
    nc.sync.dma_start(srecip[:], rs128[:])
    bcast = npool.tile([64, 512], f32, tag="bc")
    nc.gpsimd.partition_broadcast(bcast[:], srecip[:])
    nc.vector.tensor_tensor(
        oT[hp : hp + 64, hm, bass.ts(c, 512)],
        potsb[0:64, :],
        bcast[:],
        mybir.AluOpType.mult,
    )


def build_kernel():
    nc = bacc.Bacc("TRN2", target_bir_lowering=False, debug=False, num_devices=8)

    xT_d = nc.dram_tensor("xT", [C, T], f32, kind="ExternalInput").ap()
    wq_d = nc.dram_tensor("wq", [C, 512], f32, kind="ExternalInput").ap()
    wk_d = nc.dram_tensor("wk", [C, 512], f32, kind="ExternalInput").ap()
    wv_d = nc.dram_tensor("wv", [C, 512], f32, kind="ExternalInput").ap()
    wo_d = nc.dram_tensor("wo", [512, C], f32, kind="ExternalInput").ap()
    qaug_d = nc.dram_tensor("qaugb", [8, NHC, T], bf16, kind="ExternalInput").ap()
    kaug_d = nc.dram_tensor("kaugb", [8, NHC, T], bf16, kind="ExternalInput").ap()
    biasj_d = nc.dram_tensor("biasj", [P, NHC, NJB], f32, kind="ExternalInput").ap()
    y_d = nc.dram_tensor("y", [T, C], f32, kind="ExternalOutput").ap()

    xT_r = xT_d.rearrange("(cb p) t -> p cb t", p=P)  # [128, 8, 2048]
    wq_r = wq_d.rearrange("(cb p) m -> p cb m", p=P)  # [128, 8, 512]
    wk_r = wk_d.rearrange("(cb p) m -> p cb m", p=P)
    wv_r = wv_d.rearrange("(cb p) m -> p cb m", p=P)
    # [128, 4, 2, 512]: (pair m, 512-col half cc)
    wo_r = wo_d.rearrange("(mb p) (a n) -> p mb a n", p=P, a=2)
    y_r = y_d.rearrange("(tb p) c -> p tb c", p=P)  # [128, 16, 1024]

    with tile.TileContext(nc) as tc:
        with (
            tc.tile_pool(name="persist", bufs=1) as persist,
            tc.tile_pool(name="wpool", bufs=1) as wpool,
        ):
            vaug = persist.tile([P, NJB, NHC, 66], bf16)
            oT = persist.tile([P, 4, T], bf16)
            biasj = persist.tile([P, NHC, NJB], f32)
            wob = persist.tile([P, 4, 2, 512], bf16)
            xb = wpool.tile([P, 8, T], bf16, tag="xb")
            wvb = wpool.tile([P, 8, 512], bf16, tag="wvb")
            wqb = wpool.tile([P, 8, 512], bf16, tag="wqb")
            wkb = wpool.tile([P, 8, 512], bf16, tag="wkb")

            nc.gpsimd.memset(vaug[:, :, :, 64:66], 1.0)
            nc.sync.dma_start(biasj[:], biasj_d[:])

            # ---- x + weight loads (scoped so the f32 staging frees early) ----
            with tc.tile_pool(name="st32", bufs=2) as st32:
                # tck-major so pair-0's first projection chain (which
                # contracts all cb for t-chunk 0) is unblocked early
                for tck in range(NCH):
                    for cb in range(8):
                        x32 = st32.tile([P, 512], f32, tag="x32", bufs=6)
                        nc.sync.dma_start(x32[:], xT_r[:, cb, bass.ts(tck, 512)])
                        nc.vector.tensor_copy(xb[:, cb, bass.ts(tck, 512)], x32[:])
                for wsrc, wdst in ((wq_r, wqb), (wk_r, wkb), (wv_r, wvb)):
                    for half in range(2):
                        w32 = st32.tile([P, 4, 512], f32, tag="w32")
                        nc.sync.dma_start(w32[:], wsrc[:, 4 * half : 4 * half + 4, :])
                        nc.vector.tensor_copy(
                            wdst[:, 4 * half : 4 * half + 4, :], w32[:]
                        )
                for half in range(2):
                    wo32 = st32.tile([P, 4, 512], f32, tag="w32")
                    nc.sync.dma_start(
                        wo32[:].rearrange("p (a b) n -> p a b n", a=2),
                        wo_r[:, 2 * half : 2 * half + 2, :, :],
                    )
                    nc.vector.tensor_copy(
                        wob[:, 2 * half : 2 * half + 2, :, :],
                        wo32[:].rearrange("p (a b) n -> p a b n", a=2),
                    )

            # ---- interleaved projections + attention ----
            # Projection work (v and q/k pairs) is emitted as fill items
            # inside the attention jb loop so the PE streams projection
            # chains while the ACT engine catches up on exponentials.
            with (
                tc.tile_pool(name="qkst", bufs=2) as qkst,
                tc.tile_pool(name="qkT", bufs=2) as qkT,
                tc.tile_pool(name="ptd_p", bufs=8) as ptd_p,
                tc.tile_pool(name="pt1_p", bufs=10) as pt1_p,
                tc.tile_pool(name="pt2_p", bufs=10) as pt2_p,
                tc.tile_pool(name="npool", bufs=2) as npool,
                tc.tile_pool(name="psP", bufs=2, space="PSUM") as psP,
                tc.tile_pool(name="psD", bufs=3, space="PSUM") as psD,
                tc.tile_pool(name="psA2", bufs=1, space="PSUM") as psA2,
                tc.tile_pool(name="psC", bufs=1, space="PSUM") as psC,
            ):
                def v_chain(tck, tb):
                    psv = psP.tile([P, 512], f32, tag="pb")
                    for c in range(8):
                        nc.tensor.matmul(
                            psv[:],
                            xb[:, c, 512 * tck + 128 * tb : 512 * tck + 128 * (tb + 1)],
                            wvb[:, c, :],
                            start=(c == 0),
                            stop=(c == 7),
                        )
                    nc.vector.tensor_copy(
                        vaug[:, 4 * tck + tb, :, 0:64],
                        psv[:].rearrange("p (h d) -> p h d", h=NHC),
                    )

                def qk_chain(m, tck, qsg, ksg):
                    psq = psP.tile([P, 512], f32, tag="pb")
                    psk = psP.tile([P, 512], f32, tag="pb")
                    for c in range(8):
                        nc.tensor.matmul(
                            psq[:],
                            wqb[:, c, bass.ts(m, P)],
                            xb[:, c, bass.ts(tck, 512)],
                            start=(c == 0),
                            stop=(c == 7),
                        )
                        nc.tensor.matmul(
                            psk[:],
                            wkb[:, c, bass.ts(m, P)],
                            xb[:, c, bass.ts(tck, 512)],
                            start=(c == 0),
                            stop=(c == 7),
                        )
                    nc.vector.tensor_copy(qsg[:, bass.ts(tck, 512)], psq[:])
                    nc.vector.tensor_copy(ksg[:, bass.ts(tck, 512)], psk[:])

                pair_tiles = {}
                pair_state = {}

                def start_pair(m):
                    qTt = qkT.tile([72, 2, T], bf16, tag="qT")
                    kTt = qkT.tile([72, 2, T], bf16, tag="kT")
                    nc.sync.dma_start(qTt[64:72, :, :], qaug_d[:, 2 * m : 2 * m + 2, :])
                    nc.sync.dma_start(kTt[64:72, :, :], kaug_d[:, 2 * m : 2 * m + 2, :])
                    qsg = qkst.tile([P, T], bf16, tag="qsg")
                    ksg = qkst.tile([P, T], bf16, tag="ksg")
                    pair_tiles[m] = (qTt, kTt)
                    pair_state[m] = (qsg, ksg)

                def pair_item(m, tck):
                    if tck == 0:
                        start_pair(m)
                    qsg, ksg = pair_state[m]
                    qk_chain(m, tck, qsg, ksg)
                    if tck == 3:
                        qTt, kTt = pair_tiles[m]
                        nc.sync.dma_start(qTt[0:64, 0, :], qsg[0:64, :])
                        nc.sync.dma_start(qTt[0:64, 1, :], qsg[64:128, :])
                        nc.sync.dma_start(kTt[0:64, 0, :], ksg[0:64, :])
                        nc.sync.dma_start(kTt[0:64, 1, :], ksg[64:128, :])

                # pair 0 projected up front (slot 0 needs it immediately)
                for tck in range(NCH):
                    pair_item(0, tck)

                # fill items per slot: v projection streams through slot 0
                # (PV of chunk c only needs v blocks <= 4c+3); pair m's
                # projections stream through earlier slots.
                def v_item(tck, tb):
                    v_chain(tck, tb)

                fills = {s: [] for s in range(NHC)}
                fills[0] = [
                    (tb_ + 4 * tck_, lambda tck=tck_, tb=tb_: v_item(tck, tb))
                    for tck_ in range(NCH)
                    for tb_ in range(4)
                ]
                fills[1] = [
                    (4 * tck_, lambda tck=tck_: pair_item(1, tck))
                    for tck_ in range(NCH)
                ]
                for pm in (2, 3):
                    for tck_ in range(NCH):
                        sl = 2 * (pm - 1) + tck_ // 2
                        fills[sl].append(
                            (8 * (tck_ % 2), lambda tck=tck_, m=pm: pair_item(m, tck))
                        )

                # (s, jb, c) -> (tile, mid_idx or None, width, pot col offset)
                pt_reg = {}
                pend_pv = []  # chunks whose PV is deferred one jb step
                pend_norm = []  # pots whose normalization is deferred one more

                def emit_pv(s, c):
                    jmin = JBMIN[s][c]
                    njb = 4 * c + 4 - jmin
                    pot = psC.tile([65, 512], f32, tag="pot")
                    for idx, jbp in enumerate(range(jmin, 4 * c + 4)):
                        tl, mi, w, off = pt_reg.pop((s, jbp, c))
                        mov = tl[:, 0:w] if mi is None else tl[:, mi, 0:w]
                        nc.tensor.matmul(
                            pot[:, off : off + w],
                            vaug[:, jbp, s, 0:65],
                            mov,
                            start=(idx == 0),
                            stop=(idx == njb - 1),
                        )
                    pend_norm.append((s, pot, c))

                for s in range(NHC):
                    qTt, kTt = pair_tiles[s // 2]
                    sp = s % 2
                    slot_fills = list(fills[s])
                    for jb in range(NJB):
                        # projection fill items first: PE work that gives
                        # the ACT engine headroom before this jb's QK needs
                        # score-PSUM buffers back.
                        while slot_fills and slot_fills[0][0] <= jb:
                            slot_fills.pop(0)[1]()

                        c0 = jb // 4
                        r = jb % 4
                        c1 = C1[s][jb]
                        if r > 0:
                            # diagonal chunk, narrowed: cols 128r..512 of c0
                            w = 512 - 128 * r
                            sd = psD.tile([P, 512], f32, tag="sd")
                            nc.tensor.matmul(
                                sd[:, 0:w],
                                kTt[:, sp, bass.ts(jb, P)],
                                qTt[:, sp, 512 * c0 + 128 * r : 512 * (c0 + 1)],
                                start=True,
                                stop=True,
                            )
                            td = ptd_p.tile([P, 512], bf16, tag="ptd")
                            nc.scalar.activation(
                                td[:, 0:w],
                                sd[:, 0:w],
                                mybir.ActivationFunctionType.Exp,
                                bias=biasj[:, s, jb : jb + 1],
                                scale=1.0,
                            )
                            _select_diag(nc, td[:, 0:128])
                            pt_reg[(s, jb, c0)] = (td, None, w, 128 * r)
                            fulls = list(range(c0 + 1, c1 + 1))
                        else:
                            fulls = list(range(c0, c1 + 1))

                        g = 0
                        while g < len(fulls):
                            ng = min(2, len(fulls) - g)
                            if ng == 2:
                                s2 = psA2.tile([P, 2, 512], f32, tag="sa")
                                t2 = pt2_p.tile([P, 2, 512], bf16, tag="pt2")
                                for i in range(2):
                                    nc.tensor.matmul(
                                        s2[:, i, :],
                                        kTt[:, sp, bass.ts(jb, P)],
                                        qTt[:, sp, bass.ts(fulls[g + i], 512)],
                                        start=True,
                                        stop=True,
                                    )
                                nc.scalar.activation(
                                    t2[:],
                                    s2[:],
                                    mybir.ActivationFunctionType.Exp,
                                    bias=biasj[:, s, jb : jb + 1],
                                    scale=1.0,
                                )
                                for i in range(2):
                                    pt_reg[(s, jb, fulls[g + i])] = (t2, i, 512, 0)
                            else:
                                s1 = psD.tile([P, 512], f32, tag="sd")
                                t1 = pt1_p.tile([P, 512], bf16, tag="pt1")
                                nc.tensor.matmul(
                                    s1[:],
                                    kTt[:, sp, bass.ts(jb, P)],
                                    qTt[:, sp, bass.ts(fulls[g], 512)],
                                    start=True,
                                    stop=True,
                                )
                                nc.scalar.activation(
                                    t1[:],
                                    s1[:],
                                    mybir.ActivationFunctionType.Exp,
                                    bias=biasj[:, s, jb : jb + 1],
                                    scale=1.0,
                                )
                                pt_reg[(s, jb, fulls[g])] = (t1, None, 512, 0)
                            g += ng

                        if r == 0:
                            tl, mi, _, _ = pt_reg[(s, jb, c0)]
                            sel_ap = tl[:, 0:128] if mi is None else tl[:, mi, 0:128]
                            _select_diag(nc, sel_ap)

                        # flush work deferred from the previous jb step
                        if pend_norm:
                            ns_, pot_, nc_ = pend_norm.pop(0)
                            _emit_norm(nc, npool, oT, ns_, pot_, nc_)
                        if pend_pv:
                            emit_pv(*pend_pv.pop(0))
                        if (jb + 1) % 4 == 0:
                            pend_pv.append((s, jb // 4))

                    while slot_fills:
                        slot_fills.pop(0)[1]()

                # drain the tail
                while pend_pv or pend_norm:
                    if pend_norm:
                        ns_, pot_, nc_ = pend_norm.pop(0)
                        _emit_norm(nc, npool, oT, ns_, pot_, nc_)
                    if pend_pv:
                        emit_pv(*pend_pv.pop(0))

            # ---- phase D: output projection ----
            with (
                tc.tile_pool(name="ypool", bufs=4) as ypool,
                tc.tile_pool(name="psY", bufs=4, space="PSUM") as psY,
            ):
                for tb in range(NJB):
                    for cc in range(2):
                        psy = psY.tile([P, 512], f32, tag="py")
                        for m in range(4):
                            nc.tensor.matmul(
                                psy[:],
                                oT[:, m, bass.ts(tb, P)],
                                wob[:, m, cc, :],
                                start=(m == 0),
                                stop=(m == 3),
                            )
                        ysb = ypool.tile([P, 512], f32, tag="ysb")
                        nc.vector.tensor_copy(ysb[:], psy[:])
                        nc.sync.dma_start(y_r[:, tb, bass.ts(cc, 512)], ysb[:])

    nc.compile()
    return nc


def kernel(x, Wq, Wk, Wv, Wo):
    global LAST_RESULTS, _NC_CACHE
    import ml_dtypes

    x = np.asarray(x, dtype=np.float32)
    Wq = np.asarray(Wq, dtype=np.float32)
    Wk = np.asarray(Wk, dtype=np.float32)
    Wv = np.asarray(Wv, dtype=np.float32)
    Wo = np.asarray(Wo, dtype=np.float32)

    slopes = np.asarray(get_slopes(NH), dtype=np.float64)
    ii = np.arange(T, dtype=np.float64)
    pp = np.arange(P, dtype=np.float64)

    if _NC_CACHE is None:
        _NC_CACHE = build_kernel()
    nc = _NC_CACHE

    in_maps = []
    for core in range(8):
        b, g = core // 2, core % 2
        perm = list(range(g, NH, 2))  # slot s -> original head 2s+g
        core_slopes = slopes[perm]

        qaug1 = (-core_slopes[:, None] * ii[None, :]).astype(ml_dtypes.bfloat16)
        qaugb = np.ascontiguousarray(np.broadcast_to(qaug1[:, None, :], (8, NHC, T)))
        kaugb = np.zeros((8, NHC, T), ml_dtypes.bfloat16)
        for h in range(NHC):
            kaugb[h, h, :] = ml_dtypes.bfloat16(1.0)
        biasj = np.zeros((P, NHC, NJB), np.float32)
        for h in range(NHC):
            for jb in range(NJB):
                biasj[:, h, jb] = (core_slopes[h] * (128 * jb + pp)).astype(np.float32)

        wq_g = np.concatenate([Wq[:, 64 * h : 64 * h + 64] for h in perm], axis=1)
        wk_g = np.concatenate([Wk[:, 64 * h : 64 * h + 64] for h in perm], axis=1)
        wv_g = np.concatenate([Wv[:, 64 * h : 64 * h + 64] for h in perm], axis=1)
        wo_g = np.concatenate([Wo[64 * h : 64 * h + 64, :] for h in perm], axis=0)

        in_maps.append(
            {
                "xT": np.ascontiguousarray(x[b].T),
                "wq": np.ascontiguousarray(wq_g) * np.float32(0.125),
                "wk": np.ascontiguousarray(wk_g),
                "wv": np.ascontiguousarray(wv_g),
                "wo": np.ascontiguousarray(wo_g),
                "qaugb": qaugb,
                "kaugb": kaugb,
                "biasj": biasj,
            }
        )

    res = run_bass_kernel_spmd(nc, in_maps, list(range(8)))
    LAST_RESULTS = res
    out = np.empty((B, T, C), dtype=np.float32)
    for b in range(B):
        out[b] = res.results[2 * b]["y"] + res.results[2 * b + 1]["y"]
    return out
